# revision 1
# baseline (speedup 1.0000x reference)
"""ClassAlignmentLoss kernel for 8 TRN2 NeuronCores (Bass/Tile).

Data-parallel over N per domain: each core takes a contiguous 8192-sample
shard of every domain, computes local per-class segment sums/counts (one-hot
matmul on TensorE), all-reduces the [C, D+1] partials across the 8 cores,
then computes the compactness term from an SBUF-resident fp16 copy of its
feature shard (D = -F + onehotT.T @ centers via two matmuls, squared and
row-accumulated on ScalarE).  Center-distance terms are finished on host
from the (tiny, replicated) all-reduced sums.
"""

import numpy as np

# Problem shape (hardcoded per contract).
N_DOM = 3
N = 65536
D = 256
C = 64
ALPHA, BETA, GAMA = 1.0, 1.0, 1.0
N_CORES = 8
NSH = N // N_CORES          # samples per core per domain
P = 128                     # partitions / tile height


def build(nsh=NSH, n_chunks=4):
    """Build + compile the SPMD Bass module. nsh = per-core samples/domain."""
    import concourse.bass as bass
    import concourse.bacc as bacc
    import concourse.mybir as mybir
    import concourse.tile as tile

    dt = mybir.dt
    Alu = mybir.AluOpType
    Act = mybir.ActivationFunctionType

    tiles = nsh // P                    # 128-sample tiles per domain
    assert nsh % P == 0 and tiles % n_chunks == 0
    tpc = tiles // n_chunks             # tiles per DMA chunk
    half = nsh // 2                     # onehotT built in 2 half-domain chunks

    nc = bacc.Bacc(
        "TRN2",
        target_bir_lowering=False,
        debug=False,
        num_devices=N_CORES,
    )

    feat = nc.dram_tensor("feat", [N_DOM, nsh, D], dt.float32, kind="ExternalInput")
    labels = nc.dram_tensor("labels", [N_DOM, nsh], dt.int32, kind="ExternalInput")
    out_sums = nc.dram_tensor(
        "out_sums", [N_DOM, C, D + 1], dt.float32, kind="ExternalOutput"
    )
    out_comp = nc.dram_tensor("out_comp", [N_DOM, 1], dt.float32, kind="ExternalOutput")

    rg = [list(range(N_CORES))]

    with tile.TileContext(nc) as tc:
        with (
            tc.tile_pool(name="persist", bufs=1) as pp,
            tc.tile_pool(name="lab", bufs=2) as labp,
            tc.tile_pool(name="oh", bufs=3) as ohp,
            tc.tile_pool(name="ohT", bufs=1) as ohTp,
            tc.tile_pool(name="cent", bufs=2) as centp,
            tc.tile_pool(name="sq", bufs=3) as sqp,
            tc.tile_pool(name="pseg", bufs=1, space="PSUM") as psegp,
            tc.tile_pool(name="plc", bufs=1, space="PSUM") as plcp,
            tc.tile_pool(name="pcnt", bufs=1, space="PSUM") as pcntp,
            tc.tile_pool(name="pd", bufs=3, space="PSUM") as pdp,
            tc.tile_pool(name="ptrash", bufs=1, space="PSUM") as ptrashp,
            tc.tile_pool(name="pq", bufs=1, space="PSUM") as pqp,
            tc.tile_pool(name="dram", bufs=1, space="DRAM") as dramp,
        ):
            # ---- constants -------------------------------------------------
            iota64_i = pp.tile([P, C], dt.int16, tag="iota64_i")
            nc.gpsimd.iota(iota64_i[:], pattern=[[1, C]], base=0, channel_multiplier=0)
            # fp16 copy: 16-bit in/out lets the per-tile one-hot build hit 4x mode
            iota64 = pp.tile([P, C], dt.float16, tag="iota64")
            nc.vector.tensor_copy(iota64[:], iota64_i[:])
            iota128 = pp.tile([P, P], dt.int16, tag="iota128")
            nc.gpsimd.iota(iota128[:], pattern=[[1, P]], base=0, channel_multiplier=0)
            pidx_i = pp.tile([P, 1], dt.int32, tag="pidx_i")
            nc.gpsimd.iota(pidx_i[:], pattern=[[0, 1]], base=0, channel_multiplier=1)
            pidx = pp.tile([P, 1], dt.float32, tag="pidx")
            nc.vector.tensor_copy(pidx[:], pidx_i[:])
            cidx_i = pp.tile([C, 1], dt.int32, tag="cidx_i")
            nc.gpsimd.iota(cidx_i[:], pattern=[[0, 1]], base=0, channel_multiplier=1)
            cidx = pp.tile([C, 1], dt.float32, tag="cidx")
            nc.vector.tensor_copy(cidx[:], cidx_i[:])
            # negated identity (fp16): -1 where col == row
            negI = pp.tile([P, P], dt.float16, tag="negI")
            nc.vector.tensor_scalar(
                negI[:], iota128[:], pidx[:], -1.0, Alu.is_equal, Alu.mult
            )
            posI = pp.tile([P, P], dt.float16, tag="posI")
            nc.vector.tensor_scalar(
                posI[:], iota128[:], pidx[:], None, Alu.is_equal
            )
            ones_col = pp.tile([P, 1], dt.float32, tag="ones_col")
            nc.vector.memset(ones_col[:], 1.0)
            ones16 = pp.tile([P, 1], dt.float16, tag="ones16")
            nc.vector.memset(ones16[:], 1.0)

            # ---- persistent state -----------------------------------------
            # fp16 feature cache: [128, N_DOM * tiles * D]
            f16 = pp.tile([P, N_DOM * tiles * D], dt.float16, tag="f16")
            # per-sample sum-of-squares accumulator columns (one per tile)
            qsum = pqp.tile([P, N_DOM * tiles], dt.float32, tag="qsum")
            s_loc = [
                pp.tile([C, D + 1], dt.float32, tag=f"sloc{d}", name=f"sloc{d}")
                for d in range(N_DOM)
            ]
            s_glob = [
                pp.tile([C, D + 1], dt.float32, tag=f"sglob{d}", name=f"sglob{d}")
                for d in range(N_DOM)
            ]
            cnt_tmp = pp.tile([C, 2], dt.float32, tag="cnt_tmp")

            ohT = [
                ohTp.tile([C, nsh], dt.float16, tag=f"ohT{d}", name=f"ohT{d}")
                for d in range(N_DOM)
            ]

            # ================= phase 1: stream + segment sums ==============
            for d in range(N_DOM):
                dbase = d * tiles * D

                # labels: contiguous rows [64, 128], transpose on PE
                lab_rows = labp.tile([tiles, P], dt.float16, tag="lab_rows")
                nc.gpsimd.dma_start(
                    lab_rows[:], labels[d].rearrange("(t p) -> t p", p=P)
                )
                plc = plcp.tile([P, tiles], dt.float16, tag="plc")
                nc.tensor.transpose(plc[:], lab_rows[:], posI[:tiles, :tiles])
                lab_col = labp.tile([P, tiles], dt.float32, tag="lab_col")
                nc.vector.tensor_copy(lab_col[:], plc[:])

                # ---- features: cast-DMA fp32 -> fp16 into SBUF cache ------
                for k in range(n_chunks):
                    src = feat[d, k * tpc * P:(k + 1) * tpc * P, :].rearrange(
                        "(t p) m -> p t m", p=P
                    )
                    dst = f16[
                        :, dbase + k * tpc * D: dbase + (k + 1) * tpc * D
                    ].rearrange("p (t m) -> p t m", m=D)
                    nc.gpsimd.dma_start(dst, src)

                # ---- pass 1: segment sums ---------------------------------
                pseg = psegp.tile([C, D], dt.float32, tag="pseg")
                pcnt = pcntp.tile([C, 1], dt.float32, tag="pcnt")
                for t in range(tiles):
                    oh = ohp.tile([P, C], dt.float16, tag="oh")
                    nc.vector.tensor_scalar(
                        oh[:], iota64[:], lab_col[:, t:t + 1], None, Alu.is_equal
                    )
                    nc.tensor.matmul(
                        pseg[:],
                        oh[:],
                        f16[:, dbase + t * D: dbase + (t + 1) * D],
                        start=(t == 0),
                        stop=(t == tiles - 1),
                    )
                    nc.tensor.matmul(
                        pcnt[:],
                        oh[:],
                        ones16[:],
                        start=(t == 0),
                        stop=(t == tiles - 1),
                    )

                # ---- transposed one-hot + counts (2 half-domain chunks) ---
                for h in range(2):
                    lab_row = labp.tile([1, half], dt.float16, tag="lab_row")
                    nc.gpsimd.dma_start(
                        lab_row[:], labels[d:d + 1, h * half:(h + 1) * half]
                    )
                    lab_bc = labp.tile([C, half], dt.float16, tag="lab_bc")
                    nc.gpsimd.partition_broadcast(lab_bc[:], lab_row[:])
                    nc.vector.tensor_scalar(
                        ohT[d][:, h * half:(h + 1) * half],
                        lab_bc[:],
                        cidx[:],
                        None,
                        Alu.is_equal,
                    )
                nc.scalar.copy(s_loc[d][:, D:D + 1], pcnt[:])
                nc.scalar.copy(s_loc[d][:, 0:D], pseg[:])

                # ---- all-reduce the [C, D+1] partials ---------------------
                cc_in = dramp.tile([C, D + 1], dt.float32, tag=f"cc_in{d}")
                cc_out = dramp.tile([C, D + 1], dt.float32, tag=f"cc_out{d}")
                nc.sync.dma_start(cc_in[:], s_loc[d][:])
                nc.gpsimd.collective_compute(
                    "AllReduce",
                    Alu.add,
                    replica_groups=rg,
                    ins=[cc_in.opt()],
                    outs=[cc_out.opt()],
                )
                nc.sync.dma_start(s_glob[d][:], cc_out[:])
                nc.sync.dma_start(out_sums[d], cc_out[:])

            # ================= phase 2: compactness ========================
            for d in range(N_DOM):
                dbase = d * tiles * D

                cnt_cl = centp.tile([C, 1], dt.float32, tag="cnt_cl")
                nc.vector.tensor_scalar_max(cnt_cl[:], s_glob[d][:, D:D + 1], 1.0)
                inv = centp.tile([C, 1], dt.float32, tag="inv")
                nc.vector.reciprocal(inv[:], cnt_cl[:])
                cent16 = centp.tile([C, D], dt.float16, tag="cent16")
                nc.vector.tensor_scalar(
                    cent16[:], s_glob[d][:, 0:D], inv[:], None, Alu.mult
                )

                # D = -F + onehotT.T @ centers; q += D^2
                for t in range(tiles):
                    pd_t = pdp.tile([P, D], dt.float32, tag="pd")
                    nc.tensor.matmul(
                        pd_t[:],
                        negI[:],
                        f16[:, dbase + t * D: dbase + (t + 1) * D],
                        start=True,
                        stop=False,
                    )
                    nc.tensor.matmul(
                        pd_t[:],
                        ohT[d][:, t * P:(t + 1) * P],
                        cent16[:],
                        start=False,
                        stop=True,
                    )
                    qcol = qsum[:, d * tiles + t: d * tiles + t + 1]
                    if t % 6 == 0:
                        trash = ptrashp.tile([P, D], dt.float32, tag="trash")
                        nc.scalar.activation(
                            trash[:], pd_t[:], Act.Square, accum_out=qcol
                        )
                    else:
                        # ACT squares into fp16 SBUF; DVE does the row-reduce
                        sq16 = sqp.tile([P, D], dt.float16, tag="sq16")
                        nc.scalar.activation(sq16[:], pd_t[:], Act.Square)
                        dum16 = sqp.tile([P, D], dt.float16, tag="dum16")
                        nc.vector.tensor_scalar(
                            dum16[:], sq16[:], 1.0, None,
                            Alu.mult, Alu.add, accum_out=qcol,
                        )

            # ---- finale: dist = sqrt(q); per-domain partial sums ----------
            dist = pp.tile([P, N_DOM * tiles], dt.float32, tag="dist")
            nc.scalar.activation(dist[:], qsum[:], Act.Sqrt)
            dsum = pp.tile([P, N_DOM], dt.float32, tag="dsum")
            for d in range(N_DOM):
                nc.vector.reduce_sum(
                    dsum[:, d:d + 1],
                    dist[:, d * tiles:(d + 1) * tiles],
                    axis=mybir.AxisListType.X,
                )
            pc_t = plcp.tile([N_DOM, 1], dt.float32, tag="plc")
            nc.tensor.matmul(pc_t[:], dsum[:], ones_col[:], start=True, stop=True)
            comp_sb = pp.tile([N_DOM, 1], dt.float32, tag="comp_sb")
            nc.vector.tensor_copy(comp_sb[:], pc_t[:])
            nc.sync.dma_start(out_comp[:, :], comp_sb[:])

    nc.compile()
    return nc


_CACHED = {}


def _get_nc(nsh=NSH, n_chunks=4):
    key = (nsh, n_chunks)
    if key not in _CACHED:
        _CACHED[key] = build(nsh, n_chunks)
    return _CACHED[key]


def finish_host(out_maps, n_total):
    """Combine per-core outputs into the scalar loss (numpy, float64)."""
    comp_sum = np.zeros(N_DOM, dtype=np.float64)
    for m in out_maps:
        comp_sum += m["out_comp"].reshape(-1).astype(np.float64)
    comp = comp_sum / n_total

    S = out_maps[0]["out_sums"].astype(np.float64)   # [N_DOM, C, D+1]
    sums, counts = S[:, :, :D], S[:, :, D]
    centers = sums / np.maximum(counts, 1.0)[:, :, None]

    sep = np.zeros(N_DOM, dtype=np.float64)
    for d in range(N_DOM):
        cd = centers[d]
        sq = ((cd[:, None, :] - cd[None, :, :]) ** 2).sum(-1)
        dist = np.sqrt(np.maximum(sq, 0.0))
        np.fill_diagonal(dist, 0.0)
        sep[d] = dist.sum() / (C * (C - 1))

    intra = (BETA * comp.sum() - ALPHA * sep.sum()) / N_DOM
    inter = 0.0
    n_pairs = 0
    for i in range(N_DOM):
        for j in range(i + 1, N_DOM):
            inter += np.sqrt(((centers[i] - centers[j]) ** 2).sum()) / C
            n_pairs += 1
    inter /= n_pairs
    return np.float32(GAMA * intra + inter)


def shard_inputs(features, labels, nsh):
    features = np.ascontiguousarray(np.asarray(features), dtype=np.float32)
    labels = np.ascontiguousarray(np.asarray(labels), dtype=np.int32)
    in_maps = []
    for c in range(N_CORES):
        in_maps.append({
            "feat": np.ascontiguousarray(features[:, c * nsh:(c + 1) * nsh, :]),
            "labels": np.ascontiguousarray(labels[:, c * nsh:(c + 1) * nsh]),
        })
    return in_maps


def kernel(features, labels):
    from concourse.bass_utils import run_bass_kernel_spmd

    nc = _get_nc()
    in_maps = shard_inputs(features, labels, NSH)
    res = run_bass_kernel_spmd(nc, in_maps, core_ids=list(range(N_CORES)))
    return finish_host(res.results, N)



# revision 7
# speedup vs baseline: 1.1754x; 1.1754x over previous
"""ClassAlignmentLoss kernel for 8 TRN2 NeuronCores (Bass/Tile).

Data-parallel over N per domain: each core takes a contiguous 8192-sample
shard of every domain.  Phase 1 computes local per-class segment sums AND
counts in one accumulating matmul per 128-sample tile (the fp16 feature
cache carries a ones column, so the [C, D+1] partial needs no separate
count matmuls).  The [C, D+1] partials are all-reduced (fp16 payload)
across the 8 cores.  Phase 2 computes per-sample distances to the global
class centers: two matmuls per tile build (center_gather - F) in PSUM in
groups of 6 tiles, then one bulk ACT Square and one bulk DVE 3D-reduce per
group produce the per-sample squared distances.  One-hot operands are
built in bulk on DVE (broadcast access patterns), so the PE sees long
dependency-free matmul streams and can ramp to its top p-state.
Center-distance terms are finished on host from the tiny all-reduced sums.
"""

import numpy as np

# Problem shape (hardcoded per contract).
N_DOM = 3
N = 65536
D = 256
C = 64
ALPHA, BETA, GAMA = 1.0, 1.0, 1.0
N_CORES = 8
NSH = N // N_CORES          # samples per core per domain
P = 128                     # partitions / tile height


def build(nsh=NSH, n_chunks=4):
    """Build + compile the SPMD Bass module. nsh = per-core samples/domain."""
    import concourse.bass as bass
    import concourse.bacc as bacc
    import concourse.mybir as mybir
    import concourse.tile as tile

    dt = mybir.dt
    Alu = mybir.AluOpType
    Act = mybir.ActivationFunctionType

    tiles = nsh // P                    # 128-sample tiles per domain (64)
    assert nsh % P == 0 and tiles % n_chunks == 0
    tpc = tiles // n_chunks             # tiles per DMA chunk
    DW = D + 1                          # feature cols + ones column
    G = 6                               # phase-2 tiles per PSUM group

    nc = bacc.Bacc(
        "TRN2",
        target_bir_lowering=False,
        debug=False,
        num_devices=N_CORES,
    )

    feat = nc.dram_tensor("feat", [N_DOM, nsh, D], dt.float32, kind="ExternalInput")
    labels = nc.dram_tensor("labels", [N_DOM, nsh], dt.int32, kind="ExternalInput")
    out_sums = nc.dram_tensor(
        "out_sums", [N_DOM, C, DW], dt.float16, kind="ExternalOutput"
    )
    out_comp = nc.dram_tensor("out_comp", [N_DOM, 1], dt.float32, kind="ExternalOutput")

    rg = [list(range(N_CORES))]

    with tile.TileContext(nc) as tc:
        with (
            tc.tile_pool(name="persist", bufs=1) as pp,
            tc.tile_pool(name="lab", bufs=1) as labp,
            tc.tile_pool(name="ohgrp", bufs=3) as ohp,
            tc.tile_pool(name="cent", bufs=2) as centp,
            tc.tile_pool(name="sq", bufs=2) as sqp,
            tc.tile_pool(name="pseg", bufs=1, space="PSUM") as psegp,
            tc.tile_pool(name="plc", bufs=1, space="PSUM") as plcp,
            tc.tile_pool(name="pd", bufs=2, space="PSUM") as pdp,
            tc.tile_pool(name="dram", bufs=1, space="DRAM") as dramp,
        ):
            # ---- constants -------------------------------------------------
            iota64_i = pp.tile([P, C], dt.int16, tag="iota64_i")
            nc.gpsimd.iota(iota64_i[:], pattern=[[1, C]], base=0, channel_multiplier=0)
            iota64 = pp.tile([P, C], dt.float16, tag="iota64")
            nc.vector.tensor_copy(iota64[:], iota64_i[:])
            iota128 = pp.tile([P, P], dt.int16, tag="iota128")
            nc.gpsimd.iota(iota128[:], pattern=[[1, P]], base=0, channel_multiplier=0)
            pidx_i = pp.tile([P, 1], dt.int32, tag="pidx_i")
            nc.gpsimd.iota(pidx_i[:], pattern=[[0, 1]], base=0, channel_multiplier=1)
            pidx = pp.tile([P, 1], dt.float32, tag="pidx")
            nc.vector.tensor_copy(pidx[:], pidx_i[:])
            cidx_i = pp.tile([C, 1], dt.int32, tag="cidx_i")
            nc.gpsimd.iota(cidx_i[:], pattern=[[0, 1]], base=0, channel_multiplier=1)
            cidx = pp.tile([C, 1], dt.float32, tag="cidx")
            nc.vector.tensor_copy(cidx[:], cidx_i[:])
            # negated identity (fp16): -1 where col == row
            negI = pp.tile([P, P], dt.float16, tag="negI")
            nc.vector.tensor_scalar(
                negI[:], iota128[:], pidx[:], -1.0, Alu.is_equal, Alu.mult
            )
            posI = pp.tile([P, P], dt.float16, tag="posI")
            nc.vector.tensor_scalar(
                posI[:], iota128[:], pidx[:], None, Alu.is_equal
            )
            ones_col = pp.tile([P, 1], dt.float32, tag="ones_col")
            nc.vector.memset(ones_col[:], 1.0)

            # ---- persistent state -----------------------------------------
            # fp16 feature cache with a ones column per tile: [128, ND*T*257]
            f16 = pp.tile([P, N_DOM * tiles * DW], dt.float16, tag="f16")
            f16v = f16[:].rearrange("p (t m) -> p t m", m=DW)
            nc.vector.memset(f16v[:, :, D:DW], 1.0)
            # per-sample squared distances, one column per tile
            q = pp.tile([P, N_DOM * tiles], dt.float32, tag="q")
            s_loc = [
                pp.tile([C, DW], dt.float16, tag=f"sloc{d}", name=f"sloc{d}")
                for d in range(N_DOM)
            ]
            s_glob = [
                pp.tile([C, DW], dt.float16, tag=f"sglob{d}", name=f"sglob{d}")
                for d in range(N_DOM)
            ]
            lab16 = [
                pp.tile([P, tiles], dt.float16, tag=f"lab16_{d}", name=f"lab16_{d}")
                for d in range(N_DOM)
            ]

            # ---- labels: DMA + PE transpose + bulk one-hot ----------------
            for d in range(N_DOM):
                lab_rows = labp.tile([tiles, P], dt.float16, tag="lab_rows", bufs=2)
                nc.gpsimd.dma_start(
                    lab_rows[:], labels[d].rearrange("(t p) -> t p", p=P)
                )
                plc = plcp.tile([P, tiles], dt.float16, tag="plc")
                nc.tensor.transpose(plc[:], lab_rows[:], posI[:tiles, :tiles])
                nc.vector.tensor_copy(lab16[d][:], plc[:])

            oh_all = []
            for d in range(N_DOM):
                # one-hot [128, tiles*C]: oh[p, t, c] = (labels[t*128+p] == c)
                oh = ohp.tile([P, tiles * C], dt.float16, tag="ohgrp", name=f"oh{d}")
                nc.vector.tensor_tensor(
                    oh[:].rearrange("p (t c) -> p t c", c=C),
                    lab16[d][:].rearrange("p (t o) -> p t o", o=1).broadcast_to(
                        (P, tiles, C)
                    ),
                    iota64[:].rearrange("p (o c) -> p o c", o=1).broadcast_to(
                        (P, tiles, C)
                    ),
                    Alu.is_equal,
                )
                oh_all.append(oh)

            # ---- helpers ---------------------------------------------------
            def feat_dma(d):
                dbase = d * tiles * DW
                for k in range(n_chunks):
                    src = feat[d, k * tpc * P:(k + 1) * tpc * P, :].rearrange(
                        "(t p) m -> p t m", p=P
                    )
                    dst = f16[
                        :, dbase + k * tpc * DW: dbase + (k + 1) * tpc * DW
                    ].rearrange("p (t m) -> p t m", m=DW)[:, :, 0:D]
                    nc.gpsimd.dma_start(dst, src)

            def bc_labels(d):
                # broadcast labels across C partitions (gpsimd; feeds ohT)
                lab_row = labp.tile([1, nsh], dt.float16, tag="lab_row")
                nc.gpsimd.dma_start(lab_row[:], labels[d:d + 1, :])
                lab_bc = labp.tile([C, nsh], dt.float16, tag="lab_bc")
                nc.gpsimd.partition_broadcast(lab_bc[:], lab_row[:])
                return lab_bc

            ohT = [None] * N_DOM

            def build_ohT(d, lab_bc):
                # transposed one-hot [C, nsh] (reuses oh slot rotation)
                t_ohT = ohp.tile([C, nsh], dt.float16, tag="ohgrp", name=f"ohT{d}")
                nc.vector.tensor_scalar(
                    t_ohT[:], lab_bc[:], cidx[:], None, Alu.is_equal
                )
                ohT[d] = t_ohT

            def p1(d):
                # segment sums + counts: one accumulating matmul per tile
                dbase = d * tiles * DW
                pseg = psegp.tile([C, DW], dt.float32, tag="pseg")
                for t in range(tiles):
                    nc.tensor.matmul(
                        pseg[:],
                        oh_all[d][:, t * C:(t + 1) * C],
                        f16[:, dbase + t * DW: dbase + (t + 1) * DW],
                        start=(t == 0),
                        stop=(t == tiles - 1),
                    )
                nc.scalar.copy(s_loc[d][:], pseg[:])
                cc_in = dramp.tile([C, DW], dt.float16, tag=f"ccin{d}", name=f"ccin{d}")
                cc_out = dramp.tile(
                    [C, DW], dt.float16, tag=f"ccout{d}", name=f"ccout{d}"
                )
                nc.sync.dma_start(cc_in[:], s_loc[d][:])
                nc.gpsimd.collective_compute(
                    "AllReduce",
                    Alu.add,
                    replica_groups=rg,
                    ins=[cc_in.opt()],
                    outs=[cc_out.opt()],
                )
                nc.sync.dma_start(s_glob[d][:], cc_out[:])
                nc.sync.dma_start(out_sums[d], cc_out[:])

            def p2(d):
                # per-sample squared distance to global centers
                dbase = d * tiles * DW
                cnt_cl = centp.tile([C, 1], dt.float32, tag="cnt_cl")
                nc.vector.tensor_scalar_max(cnt_cl[:], s_glob[d][:, D:DW], 1.0)
                inv = centp.tile([C, 1], dt.float32, tag="inv")
                nc.vector.reciprocal(inv[:], cnt_cl[:])
                cent16 = centp.tile([C, D], dt.float16, tag="cent16")
                nc.vector.tensor_scalar(
                    cent16[:], s_glob[d][:, 0:D], inv[:], None, Alu.mult
                )

                t0 = 0
                while t0 < tiles:
                    g = min(G, tiles - t0)
                    pd_t = pdp.tile([P, G * D], dt.float32, tag="pd")
                    for j in range(g):
                        t = t0 + j
                        dst = pd_t[:, j * D:(j + 1) * D]
                        nc.tensor.matmul(
                            dst,
                            negI[:],
                            f16[:, dbase + t * DW: dbase + t * DW + D],
                            start=True,
                            stop=False,
                        )
                        nc.tensor.matmul(
                            dst,
                            ohT[d][:, t * P:(t + 1) * P],
                            cent16[:],
                            start=False,
                            stop=True,
                        )
                    sq16 = sqp.tile([P, G * D], dt.float16, tag="sq")
                    nc.scalar.activation(
                        sq16[:, 0:g * D], pd_t[:, 0:g * D], Act.Square
                    )
                    nc.vector.tensor_reduce(
                        q[:, d * tiles + t0: d * tiles + t0 + g],
                        sq16[:, 0:g * D].rearrange("p (g m) -> p g m", m=D),
                        axis=mybir.AxisListType.X,
                        op=Alu.add,
                    )
                    t0 += g

            # ---- emission schedule ----------------------------------------
            feat_dma(0)
            bc0 = bc_labels(0)
            feat_dma(1)
            p1(0)
            build_ohT(0, bc0)
            feat_dma(2)
            bc1 = bc_labels(1)
            p1(1)
            build_ohT(1, bc1)
            bc2 = bc_labels(2)
            p1(2)
            p2(0)
            build_ohT(2, bc2)
            p2(1)
            p2(2)

            # ---- finale: dist = sqrt(q); per-domain partial sums ----------
            dist = pp.tile([P, N_DOM * tiles], dt.float32, tag="dist")
            nc.scalar.activation(dist[:], q[:], Act.Sqrt)
            dsum = pp.tile([P, N_DOM], dt.float32, tag="dsum")
            nc.vector.tensor_reduce(
                dsum[:],
                dist[:].rearrange("p (d t) -> p d t", t=tiles),
                axis=mybir.AxisListType.X,
                op=Alu.add,
            )
            pc_t = plcp.tile([N_DOM, 1], dt.float32, tag="plc")
            nc.tensor.matmul(pc_t[:], dsum[:], ones_col[:], start=True, stop=True)
            comp_sb = pp.tile([N_DOM, 1], dt.float32, tag="comp_sb")
            nc.vector.tensor_copy(comp_sb[:], pc_t[:])
            nc.sync.dma_start(out_comp[:, :], comp_sb[:])

    nc.compile()
    return nc


_CACHED = {}


def _get_nc(nsh=NSH, n_chunks=4):
    key = (nsh, n_chunks)
    if key not in _CACHED:
        _CACHED[key] = build(nsh, n_chunks)
    return _CACHED[key]


def finish_host(out_maps, n_total):
    """Combine per-core outputs into the scalar loss (numpy, float64)."""
    comp_sum = np.zeros(N_DOM, dtype=np.float64)
    for m in out_maps:
        comp_sum += m["out_comp"].reshape(-1).astype(np.float64)
    comp = comp_sum / n_total

    S = out_maps[0]["out_sums"].astype(np.float64)   # [N_DOM, C, D+1]
    sums, counts = S[:, :, :D], S[:, :, D]
    centers = sums / np.maximum(counts, 1.0)[:, :, None]

    sep = np.zeros(N_DOM, dtype=np.float64)
    for d in range(N_DOM):
        cd = centers[d]
        sq = ((cd[:, None, :] - cd[None, :, :]) ** 2).sum(-1)
        dist = np.sqrt(np.maximum(sq, 0.0))
        np.fill_diagonal(dist, 0.0)
        sep[d] = dist.sum() / (C * (C - 1))

    intra = (BETA * comp.sum() - ALPHA * sep.sum()) / N_DOM
    inter = 0.0
    n_pairs = 0
    for i in range(N_DOM):
        for j in range(i + 1, N_DOM):
            inter += np.sqrt(((centers[i] - centers[j]) ** 2).sum()) / C
            n_pairs += 1
    inter /= n_pairs
    return np.float32(GAMA * intra + inter)


def shard_inputs(features, labels, nsh):
    features = np.ascontiguousarray(np.asarray(features), dtype=np.float32)
    labels = np.ascontiguousarray(np.asarray(labels), dtype=np.int32)
    in_maps = []
    for c in range(N_CORES):
        in_maps.append({
            "feat": np.ascontiguousarray(features[:, c * nsh:(c + 1) * nsh, :]),
            "labels": np.ascontiguousarray(labels[:, c * nsh:(c + 1) * nsh]),
        })
    return in_maps


def kernel(features, labels):
    from concourse.bass_utils import run_bass_kernel_spmd

    nc = _get_nc()
    in_maps = shard_inputs(features, labels, NSH)
    res = run_bass_kernel_spmd(nc, in_maps, core_ids=list(range(N_CORES)))
    return finish_host(res.results, N)
